# revision 37
# baseline (speedup 1.0000x reference)
"""LiteMLA (linear attention) Trainium2 kernel — fp8 DoubleRow edition.

Full-input contract: kernel(**inputs) takes the unsharded tensors from
setup_inputs() and returns the full (16, 256, 64, 64) float32 output.

Strategy
--------
Data-parallel over batch: 16 batch elements -> 8 NeuronCores x 2 each.
All heavy matmuls run in fp8e4m3 with DoubleRow perf mode (two 128-deep
k-tiles contracted per pass = 157 TF/s, 2x bf16). The projection matrix
is folded into the V weights on the host (v' = (Wp'·Wv)·x with
Wp' = diag(bn_scale)·w_proj), so the kv state IS the projected state:

  K phase   one DR matmul per 128-wide n-chunk against the combined
            [Wk | Wp'Wv] weight block -> PSUM [128, 512] = [kt | vt'];
            relu-cast kt -> fp8, cast vt' -> fp8 (ones column prefilled)
  Q phase   q[c,n] = relu(Wq x) via weight-stationary DR matmuls -> fp8
            (emitted between K and kv so the PE never waits on casts)
  kv phase  M[c,o] = sum_n kt[n,c] vt'[n,o]  (DR over n-pairs, PSUM f32)
            column 256 = ksum (via the ones column); a single cast
            M8 = fp8(M * S) with S = 1/64 produces the Z-phase operand
  Z phase   z[n, 0:257] = q-chunk-stationary DR matmul against M8;
            column 256 is the denominator (the S scaling cancels in the
            ratio). Chunk pairs share a 2-bank PSUM tile so one bf16
            cast moves both; groups of 8 chunks ship per DMA.

The device ships z[b, p, i, 0:257] bf16 (n = i*128+p); the host does
y = z[:, :256]/z[:, 256] + BN bias and transposes back to
(B, C, H, W). eps=1e-5 is negligible against den ~ O(1e3).
"""

import numpy as np
import ml_dtypes

import concourse.bass as bass
from concourse import bacc
import concourse.mybir as mybir
import concourse.tile as tile
from concourse.bass_utils import run_bass_kernel_spmd

B, C, H, W = 16, 256, 64, 64
N = H * W            # 4096
NCORES = 8
BL = B // NCORES     # batch elements per core
NT = N // 128        # 32 n-chunks
NPAIR = NT // 2      # 16 n-pair chunks
NTW = N // 512       # 8 wide n-tiles for q
NGRP = NT // 8       # 4 z-DMA groups of 8 chunks
S = 1.0 / 64.0       # fp8 state scale (cancels in z/den)

BF16 = mybir.dt.bfloat16
F32 = mybir.dt.float32
FP8 = mybir.dt.float8e4
NPBF16 = ml_dtypes.bfloat16
NPFP8 = ml_dtypes.float8_e4m3

Q_ACT_OF_8 = 4       # of every 8 q tiles, this many go to ACT
Z_ACT_OF_8 = 4       # of every 8 z pairs, this many go to ACT

_CACHE = {}


def _build_program():
    nc = bacc.Bacc("TRN2", target_bir_lowering=False, debug=False)

    xs = nc.dram_tensor("x8", [BL, 128, 2, N], FP8, kind="ExternalInput")
    wq = nc.dram_tensor("wq8", [128, 2, C], FP8, kind="ExternalInput")
    wkv = nc.dram_tensor("wkv8", [128, 2, 2 * C], FP8, kind="ExternalInput")
    zs = nc.dram_tensor("z", [BL, 128, NT, C + 1], BF16, kind="ExternalOutput")

    Relu = mybir.ActivationFunctionType.Relu
    Copy = mybir.ActivationFunctionType.Copy
    DR = mybir.MatmulPerfMode.DoubleRow

    with tile.TileContext(nc) as tc:
        with (
            tc.tile_pool(name="const", bufs=1) as cp,
            tc.tile_pool(name="xp", bufs=2) as xp,
            tc.tile_pool(name="qp", bufs=2) as qp,
            tc.tile_pool(name="ktp", bufs=1) as ktp,
            tc.tile_pool(name="vtp", bufs=1) as vtp,
            tc.tile_pool(name="small", bufs=2) as sp,
            tc.tile_pool(name="hout", bufs=3) as hp,
            tc.tile_pool(name="ps_big", bufs=4, space="PSUM") as ps_big,
        ):
            # ---------- weights first (small, K needs wkv), then x ----------
            wq_sb = cp.tile([128, 2, C], FP8, tag="wq", name="wq")
            wkv_sb = cp.tile([128, 2, 2 * C], FP8, tag="wkv", name="wkv")
            nc.sync.dma_start(out=wkv_sb[:], in_=wkv[:])
            nc.sync.dma_start(out=wq_sb[:], in_=wq[:])
            x_sb = {}
            x_sb[0] = xp.tile([128, 2, N], FP8, tag="x", name="x_0")
            xsl = [(0, 512), (512, 1536), (2048, 1024), (3072, 1024)]
            for s0, slen in xsl:
                nc.sync.dma_start(out=x_sb[0][:, :, s0:s0 + slen],
                                  in_=xs[0, :, :, s0:s0 + slen])

            # kt/vt staging tiles, shared across batches; ones columns
            # of vt are written once here and never touched again
            kt8 = [ktp.tile([128, 2, C], FP8, tag=f"kt{p}", name=f"kt_{p}")
                   for p in range(NPAIR)]
            vt8 = [vtp.tile([128, 2, C + 1], FP8, tag=f"vt{p}", name=f"vt_{p}")
                   for p in range(NPAIR)]
            for p in range(NPAIR):
                for j in range(2):
                    nc.gpsimd.memset(vt8[p][:, j, C:C + 1], 1.0)

            # Z-phase pair emitter (software-pipelined: Z of batch b-1 is
            # zipped into K of batch b so the cast engines never idle)
            zstate = {}

            def emit_z_pair(zb, q8z, m8z, p, glen=8):
                if p % (glen // 2) == 0:
                    zstate["sb"] = hp.tile([128, glen, C + 1], BF16, tag="z",
                                           name=f"z_{zb}_{p}")
                z_sb = zstate["sb"]
                jp = p % (glen // 2)
                zps = ps_big.tile([128, 2, 512], F32, tag="big", name=f"zps_{zb}_{p}")
                for j in range(2):
                    nc.tensor.matmul(zps[:, j, 0:C + 1],
                                     lhsT=q8z[:, :, (2 * p + j) * 128:(2 * p + j + 1) * 128],
                                     rhs=m8z[:],
                                     start=True, stop=True, perf_mode=DR)
                if p % 2 == 0:
                    nc.scalar.activation(z_sb[:, 2 * jp:2 * jp + 2, :],
                                         zps[:, :, 0:C + 1], Copy)
                else:
                    nc.vector.tensor_copy(z_sb[:, 2 * jp:2 * jp + 2, :],
                                          zps[:, :, 0:C + 1])
                if jp == glen // 2 - 1:
                    g0 = p - jp
                    nc.sync.dma_start(out=zs[zb, :, 2 * g0:2 * g0 + glen, :],
                                      in_=z_sb[:])

            prev = None  # (q8, m8) of the previous batch
            for b in range(BL):
                # ---------- K phase: combined [kt|vt'] matmuls, pair tiles --
                # zipped with Z pairs of the previous batch
                for p in range(NPAIR):
                    kvps = ps_big.tile([128, 2, 512], F32, tag="big", name=f"kvps_{b}_{p}")
                    for j in range(2):
                        nc.tensor.matmul(kvps[:, j, :],
                                         lhsT=x_sb[b][:, :, (2 * p + j) * 128:(2 * p + j + 1) * 128],
                                         rhs=wkv_sb[:], start=True, stop=True,
                                         perf_mode=DR)
                    # one merged cast per pair per engine
                    nc.scalar.activation(kt8[p][:, :, :], kvps[:, :, 0:C], Relu)
                    nc.vector.tensor_copy(vt8[p][:, :, 0:C], kvps[:, :, C:2 * C])
                    if prev is not None:
                        emit_z_pair(b - 1, prev[0], prev[1], p)

                # prefetch next batch's x during this batch's compute
                if b + 1 < BL:
                    x_sb[b + 1] = xp.tile([128, 2, N], FP8, tag="x", name=f"x_{b + 1}")
                    nc.sync.dma_start(out=x_sb[b + 1][:], in_=xs[b + 1])

                # ---------- Q phase (PE keeps running while K casts drain) ----
                q8 = qp.tile([128, 2, N], FP8, tag="q", name=f"q_{b}")
                qi = 0
                for mc in range(2):
                    for iw2 in range(NTW // 2):
                        nsl = slice(iw2 * 1024, (iw2 + 1) * 1024)
                        qps = ps_big.tile([128, 2, 512], F32, tag="big",
                                          name=f"qps_{b}_{mc}_{iw2}")
                        for j in range(2):
                            nc.tensor.matmul(
                                qps[:, j, :],
                                lhsT=wq_sb[:, :, mc * 128:(mc + 1) * 128],
                                rhs=x_sb[b][:, :, (iw2 * 2 + j) * 512:(iw2 * 2 + j + 1) * 512],
                                start=True, stop=True, perf_mode=DR)
                        if qi % 2 == 0:
                            nc.scalar.activation(q8[:, mc, nsl], qps[:], Relu)
                        else:
                            nc.vector.tensor_scalar_max(q8[:, mc, nsl], qps[:], 0.0)
                        qi += 1

                # ---------- kv accumulation: M = [MT | ksum] directly -------
                kv_ps = ps_big.tile([128, 2, 512], F32, tag="big", name=f"kv_{b}")
                for p in range(NPAIR):
                    for cc in range(2):
                        nc.tensor.matmul(kv_ps[:, cc, 0:C + 1],
                                         lhsT=kt8[p][:, :, cc * 128:(cc + 1) * 128],
                                         rhs=vt8[p][:],
                                         start=(p == 0), stop=(p == NPAIR - 1),
                                         perf_mode=DR)
                m8 = sp.tile([128, 2, C + 1], FP8, tag="m8", name=f"m8_{b}")
                nc.scalar.activation(m8[:], kv_ps[:, :, 0:C + 1], Copy, scale=S)
                prev = (q8, m8)

            # ---------- final batch's Z phase, tapered groups ----------
            for p in range(NPAIR):
                emit_z_pair(BL - 1, prev[0], prev[1], p, glen=4)
    nc.compile()
    return nc


def _prep_inputs(x, w_qkv, w_proj, bn_gamma, bn_beta, bn_mean, bn_var):
    x = np.asarray(x, dtype=np.float32)
    w_qkv = np.asarray(w_qkv, dtype=np.float32)
    w_proj = np.asarray(w_proj, dtype=np.float32)
    bn_gamma = np.asarray(bn_gamma, dtype=np.float32)
    bn_beta = np.asarray(bn_beta, dtype=np.float32)
    bn_mean = np.asarray(bn_mean, dtype=np.float32)
    bn_var = np.asarray(bn_var, dtype=np.float32)

    # torch-faithful interleave: out-channel 3*i+j -> (channel i, {q,k,v}[j])
    def w8(wm):  # (C_out, C_in) -> [128, 2, C_out] fp8: [p, j, o] = w[o, j*128+p]
        return np.ascontiguousarray(
            wm.T.reshape(2, 128, -1).transpose(1, 0, 2).astype(NPFP8))

    scale = bn_gamma / np.sqrt(bn_var + 1e-5)
    wq8 = w8(w_qkv[0::3])
    # combined [wk | Wp'·Wv] along the output dim (projection folded into V)
    wpv = (scale[:, None] * w_proj) @ w_qkv[2::3]
    wkv8 = w8(np.concatenate([w_qkv[1::3], wpv], axis=0))
    # x: (B, C, N) -> [B, 128, 2, N] fp8: [b, p, j, n] = x[b, j*128+p, n]
    x8 = np.ascontiguousarray(
        x.reshape(B, 2, 128, N).transpose(0, 2, 1, 3).astype(NPFP8))

    bias = (bn_beta - bn_mean * scale).astype(np.float32)

    in_maps = []
    for core in range(NCORES):
        in_maps.append({
            "x8": x8[core * BL:(core + 1) * BL],
            "wq8": wq8, "wkv8": wkv8,
        })
    return in_maps, bias


def _postprocess(z_raw, bias):
    # z_raw: (B, 128, NT, C+1), n = i*128+p -> y (B, C, H, W) f32
    z = z_raw.transpose(0, 2, 1, 3).reshape(B, N, C + 1)
    y = z[:, :, :C] / z[:, :, C:C + 1] + bias[None, None, :]
    return np.ascontiguousarray(y.transpose(0, 2, 1)).reshape(B, C, H, W)


def _run(inputs, trace=False, **kw):
    if "nc" not in _CACHE:
        _CACHE["nc"] = _build_program()
    nc = _CACHE["nc"]
    in_maps, bias = _prep_inputs(**inputs)
    res = run_bass_kernel_spmd(nc, in_maps, list(range(NCORES)), trace=trace, **kw)
    z_raw = np.concatenate([res.results[i]["z"] for i in range(NCORES)], axis=0)
    return _postprocess(z_raw.astype(np.float32), bias), res


def kernel(**inputs):
    y, _ = _run(inputs)
    return y


# revision 38
# speedup vs baseline: 1.0382x; 1.0382x over previous
"""LiteMLA (linear attention) Trainium2 kernel — fp8 DoubleRow edition.

Full-input contract: kernel(**inputs) takes the unsharded tensors from
setup_inputs() and returns the full (16, 256, 64, 64) float32 output.

Strategy
--------
Data-parallel over batch: 16 batch elements -> 8 NeuronCores x 2 each.
All heavy matmuls run in fp8e4m3 with DoubleRow perf mode (two 128-deep
k-tiles contracted per pass = 157 TF/s, 2x bf16). The projection matrix
is folded into the V weights on the host (v' = (Wp'·Wv)·x with
Wp' = diag(bn_scale)·w_proj), so the kv state IS the projected state:

  K phase   one DR matmul per 128-wide n-chunk against the combined
            [Wk | Wp'Wv] weight block -> PSUM [128, 512] = [kt | vt'];
            relu-cast kt -> fp8, cast vt' -> fp8 (ones column prefilled)
  Q phase   q[c,n] = relu(Wq x) via weight-stationary DR matmuls -> fp8
            (emitted between K and kv so the PE never waits on casts)
  kv phase  M[c,o] = sum_n kt[n,c] vt'[n,o]  (DR over n-pairs, PSUM f32)
            column 256 = ksum (via the ones column); a single cast
            M8 = fp8(M * S) with S = 1/64 produces the Z-phase operand
  Z phase   z[n, 0:257] = q-chunk-stationary DR matmul against M8;
            column 256 is the denominator (the S scaling cancels in the
            ratio). Chunk pairs share a 2-bank PSUM tile so one bf16
            cast moves both; groups of 8 chunks ship per DMA.

The device ships z[b, p, i, 0:257] bf16 (n = i*128+p); the host does
y = z[:, :256]/z[:, 256] + BN bias and transposes back to
(B, C, H, W). eps=1e-5 is negligible against den ~ O(1e3).
"""

import numpy as np
import ml_dtypes

import concourse.bass as bass
from concourse import bacc
import concourse.mybir as mybir
import concourse.tile as tile
from concourse.bass_utils import run_bass_kernel_spmd

B, C, H, W = 16, 256, 64, 64
N = H * W            # 4096
NCORES = 8
BL = B // NCORES     # batch elements per core
NT = N // 128        # 32 n-chunks
NPAIR = NT // 2      # 16 n-pair chunks
NTW = N // 512       # 8 wide n-tiles for q
NGRP = NT // 8       # 4 z-DMA groups of 8 chunks
S = 1.0 / 64.0       # fp8 state scale (cancels in z/den)

BF16 = mybir.dt.bfloat16
F32 = mybir.dt.float32
FP8 = mybir.dt.float8e4
NPBF16 = ml_dtypes.bfloat16
NPFP8 = ml_dtypes.float8_e4m3

Q_ACT_OF_8 = 4       # of every 8 q tiles, this many go to ACT
Z_ACT_OF_8 = 4       # of every 8 z pairs, this many go to ACT

_CACHE = {}


def _build_program():
    nc = bacc.Bacc("TRN2", target_bir_lowering=False, debug=False)

    xs = nc.dram_tensor("x8", [BL, 128, 2, N], FP8, kind="ExternalInput")
    wq = nc.dram_tensor("wq8", [128, 2, C], FP8, kind="ExternalInput")
    wkv = nc.dram_tensor("wkv8", [128, 2, 2 * C], FP8, kind="ExternalInput")
    zs = nc.dram_tensor("z", [BL, 128, NT, C + 1], BF16, kind="ExternalOutput")

    Relu = mybir.ActivationFunctionType.Relu
    Copy = mybir.ActivationFunctionType.Copy
    DR = mybir.MatmulPerfMode.DoubleRow

    with tile.TileContext(nc) as tc:
        with (
            tc.tile_pool(name="const", bufs=1) as cp,
            tc.tile_pool(name="xp", bufs=2) as xp,
            tc.tile_pool(name="qp", bufs=2) as qp,
            tc.tile_pool(name="ktp", bufs=1) as ktp,
            tc.tile_pool(name="vtp", bufs=1) as vtp,
            tc.tile_pool(name="small", bufs=2) as sp,
            tc.tile_pool(name="hout", bufs=3) as hp,
            tc.tile_pool(name="ps_big", bufs=4, space="PSUM") as ps_big,
        ):
            # ---------- first x slice + wkv first; wq deferred (Q is late) --
            wq_sb = cp.tile([128, 2, C], FP8, tag="wq", name="wq")
            wkv_sb = cp.tile([128, 2, 2 * C], FP8, tag="wkv", name="wkv")
            x_sb = {}
            x_sb[0] = xp.tile([128, 2, N], FP8, tag="x", name="x_0")
            nc.sync.dma_start(out=x_sb[0][:, :, 0:512], in_=xs[0, :, :, 0:512])
            nc.sync.dma_start(out=wkv_sb[:], in_=wkv[:])
            for s0, slen in [(512, 1536), (2048, 1024), (3072, 1024)]:
                nc.sync.dma_start(out=x_sb[0][:, :, s0:s0 + slen],
                                  in_=xs[0, :, :, s0:s0 + slen])
            nc.sync.dma_start(out=wq_sb[:], in_=wq[:])

            # kt/vt staging tiles, shared across batches; ones columns
            # of vt are written once here and never touched again
            kt8 = [ktp.tile([128, 2, C], FP8, tag=f"kt{p}", name=f"kt_{p}")
                   for p in range(NPAIR)]
            vt8 = [vtp.tile([128, 2, C + 1], FP8, tag=f"vt{p}", name=f"vt_{p}")
                   for p in range(NPAIR)]
            for p in range(NPAIR):
                for j in range(2):
                    nc.gpsimd.memset(vt8[p][:, j, C:C + 1], 1.0)

            # Z-phase pair emitter (software-pipelined: Z of batch b-1 is
            # zipped into K of batch b so the cast engines never idle)
            zstate = {}

            def emit_z_pair(zb, q8z, m8z, p, glen=8):
                if p % (glen // 2) == 0:
                    zstate["sb"] = hp.tile([128, glen, C + 1], BF16, tag="z",
                                           name=f"z_{zb}_{p}")
                z_sb = zstate["sb"]
                jp = p % (glen // 2)
                zps = ps_big.tile([128, 2, 512], F32, tag="big", name=f"zps_{zb}_{p}")
                for j in range(2):
                    nc.tensor.matmul(zps[:, j, 0:C + 1],
                                     lhsT=q8z[:, :, (2 * p + j) * 128:(2 * p + j + 1) * 128],
                                     rhs=m8z[:],
                                     start=True, stop=True, perf_mode=DR)
                if p % 2 == 0:
                    nc.scalar.activation(z_sb[:, 2 * jp:2 * jp + 2, :],
                                         zps[:, :, 0:C + 1], Copy)
                else:
                    nc.vector.tensor_copy(z_sb[:, 2 * jp:2 * jp + 2, :],
                                          zps[:, :, 0:C + 1])
                if jp == glen // 2 - 1:
                    g0 = p - jp
                    nc.sync.dma_start(out=zs[zb, :, 2 * g0:2 * g0 + glen, :],
                                      in_=z_sb[:])

            prev = None  # (q8, m8) of the previous batch
            for b in range(BL):
                # ---------- K phase: combined [kt|vt'] matmuls, pair tiles --
                # zipped with Z pairs of the previous batch
                for p in range(NPAIR):
                    kvps = ps_big.tile([128, 2, 512], F32, tag="big", name=f"kvps_{b}_{p}")
                    for j in range(2):
                        nc.tensor.matmul(kvps[:, j, :],
                                         lhsT=x_sb[b][:, :, (2 * p + j) * 128:(2 * p + j + 1) * 128],
                                         rhs=wkv_sb[:], start=True, stop=True,
                                         perf_mode=DR)
                    # one merged cast per pair per engine
                    nc.scalar.activation(kt8[p][:, :, :], kvps[:, :, 0:C], Relu)
                    nc.vector.tensor_copy(vt8[p][:, :, 0:C], kvps[:, :, C:2 * C])
                    if prev is not None:
                        emit_z_pair(b - 1, prev[0], prev[1], p)

                # prefetch next batch's x during this batch's compute
                if b + 1 < BL:
                    x_sb[b + 1] = xp.tile([128, 2, N], FP8, tag="x", name=f"x_{b + 1}")
                    nc.sync.dma_start(out=x_sb[b + 1][:], in_=xs[b + 1])

                # ---------- kv accumulation zipped with Q phase -------------
                # M = [MT | ksum] directly; Q pairs interleave so the cast
                # engines stay fed while the PE grinds the kv contraction
                q8 = qp.tile([128, 2, N], FP8, tag="q", name=f"q_{b}")

                def emit_q_pair(qi):
                    mc, iw2 = qi // (NTW // 2), qi % (NTW // 2)
                    nsl = slice(iw2 * 1024, (iw2 + 1) * 1024)
                    qps = ps_big.tile([128, 2, 512], F32, tag="big",
                                      name=f"qps_{b}_{mc}_{iw2}")
                    for j in range(2):
                        nc.tensor.matmul(
                            qps[:, j, :],
                            lhsT=wq_sb[:, :, mc * 128:(mc + 1) * 128],
                            rhs=x_sb[b][:, :, (iw2 * 2 + j) * 512:(iw2 * 2 + j + 1) * 512],
                            start=True, stop=True, perf_mode=DR)
                    if qi % 2 == 0:
                        nc.scalar.activation(q8[:, mc, nsl], qps[:], Relu)
                    else:
                        nc.vector.tensor_scalar_max(q8[:, mc, nsl], qps[:], 0.0)

                kv_ps = ps_big.tile([128, 2, 512], F32, tag="big", name=f"kv_{b}")
                for p in range(NPAIR):
                    for cc in range(2):
                        nc.tensor.matmul(kv_ps[:, cc, 0:C + 1],
                                         lhsT=kt8[p][:, :, cc * 128:(cc + 1) * 128],
                                         rhs=vt8[p][:],
                                         start=(p == 0), stop=(p == NPAIR - 1),
                                         perf_mode=DR)
                    if p % 2 == 1:
                        emit_q_pair(p // 2)
                m8 = sp.tile([128, 2, C + 1], FP8, tag="m8", name=f"m8_{b}")
                nc.scalar.activation(m8[:], kv_ps[:, :, 0:C + 1], Copy, scale=S)
                prev = (q8, m8)

            # ---------- final batch's Z phase, tapered groups ----------
            for p in range(NPAIR):
                emit_z_pair(BL - 1, prev[0], prev[1], p, glen=4)
    nc.compile()
    return nc


def _prep_inputs(x, w_qkv, w_proj, bn_gamma, bn_beta, bn_mean, bn_var):
    x = np.asarray(x, dtype=np.float32)
    w_qkv = np.asarray(w_qkv, dtype=np.float32)
    w_proj = np.asarray(w_proj, dtype=np.float32)
    bn_gamma = np.asarray(bn_gamma, dtype=np.float32)
    bn_beta = np.asarray(bn_beta, dtype=np.float32)
    bn_mean = np.asarray(bn_mean, dtype=np.float32)
    bn_var = np.asarray(bn_var, dtype=np.float32)

    # torch-faithful interleave: out-channel 3*i+j -> (channel i, {q,k,v}[j])
    def w8(wm):  # (C_out, C_in) -> [128, 2, C_out] fp8: [p, j, o] = w[o, j*128+p]
        return np.ascontiguousarray(
            wm.T.reshape(2, 128, -1).transpose(1, 0, 2).astype(NPFP8))

    scale = bn_gamma / np.sqrt(bn_var + 1e-5)
    wq8 = w8(w_qkv[0::3])
    # combined [wk | Wp'·Wv] along the output dim (projection folded into V)
    wpv = (scale[:, None] * w_proj) @ w_qkv[2::3]
    wkv8 = w8(np.concatenate([w_qkv[1::3], wpv], axis=0))
    # x: (B, C, N) -> [B, 128, 2, N] fp8: [b, p, j, n] = x[b, j*128+p, n]
    x8 = np.ascontiguousarray(
        x.reshape(B, 2, 128, N).transpose(0, 2, 1, 3).astype(NPFP8))

    bias = (bn_beta - bn_mean * scale).astype(np.float32)

    in_maps = []
    for core in range(NCORES):
        in_maps.append({
            "x8": x8[core * BL:(core + 1) * BL],
            "wq8": wq8, "wkv8": wkv8,
        })
    return in_maps, bias


def _postprocess(z_raw, bias):
    # z_raw: (B, 128, NT, C+1), n = i*128+p -> y (B, C, H, W) f32
    z = z_raw.transpose(0, 2, 1, 3).reshape(B, N, C + 1)
    y = z[:, :, :C] / z[:, :, C:C + 1] + bias[None, None, :]
    return np.ascontiguousarray(y.transpose(0, 2, 1)).reshape(B, C, H, W)


def _run(inputs, trace=False, **kw):
    if "nc" not in _CACHE:
        _CACHE["nc"] = _build_program()
    nc = _CACHE["nc"]
    in_maps, bias = _prep_inputs(**inputs)
    res = run_bass_kernel_spmd(nc, in_maps, list(range(NCORES)), trace=trace, **kw)
    z_raw = np.concatenate([res.results[i]["z"] for i in range(NCORES)], axis=0)
    return _postprocess(z_raw.astype(np.float32), bias), res


def kernel(**inputs):
    y, _ = _run(inputs)
    return y
